# revision 15
# baseline (speedup 1.0000x reference)
"""Multi-head attention (B=2, M=N=2048, D=1024, H=16, DH=64) on 8 TRN2 cores.

Sharding: data-parallel over batch (cores 0-3 = batch 0, 4-7 = batch 1),
tensor-parallel over heads within each batch group (4 heads/core).
Per core:
  - PE-transpose queries/keys/values (chan on partitions)
  - f32r projections: Q^T, K^T (64-chan head rows on partitions, pair-packed
    into two 128-partition tiles), V natural (seq on partitions) in an
    extended lhsT layout [ones | zeros | V] (softmax row-sum trick)
  - attention per (head, m-chunk): S^T = K_h Q_h^T per 128-row n-tile ->
    exp (ScalarE, f32r out) -> O^T accumulation in PSUM via lhsT=[1|0|V]
    (partition 0 = softmax denominators, partitions 64..127 = O^T rows);
    normalize with reciprocal + gpsimd partition-broadcast
  - AllGather O^T shards (4-core groups) via internal DRAM bounce
  - output projection: each core computes a 256-wide output-channel slice
    (out^T layout) using its host-sliced Wo^T columns; bias folded in.
Host folds bv/bo into one effective bias (bo + Wo @ bv) and drops bk
(row-constant logit shifts cancel in softmax).
"""

import os

import numpy as np

B, M, NSEQ, D = 2, 2048, 2048, 1024
H, DH = 16, 64
HC = 4                # heads per core
PC = HC * DH          # 256 projected channels per core
CT = D // 128         # 8 contraction tiles
NT = NSEQ // 128      # 16 n-tiles
MT = M // 512         # 4 m-chunks
NCORES = 8

_CACHE = {}


def _build(single_core=False, reps=1):
    import concourse.bass as bass
    import concourse.tile as tile
    from concourse import bacc, mybir
    from concourse.masks import make_identity

    F32 = mybir.dt.float32
    F32R = mybir.dt.float32r
    AF = mybir.ActivationFunctionType

    nc = bacc.Bacc(
        "TRN2",
        target_bir_lowering=False,
        debug=False,
        num_devices=1 if single_core else 8,
    )

    xq_d = nc.dram_tensor("xq", [M, D], F32, kind="ExternalInput")
    xk_d = nc.dram_tensor("xk", [NSEQ, D], F32, kind="ExternalInput")
    xv_d = nc.dram_tensor("xv", [NSEQ, D], F32, kind="ExternalInput")
    wqT_d = nc.dram_tensor("wqT", [D, PC], F32, kind="ExternalInput")
    wkT_d = nc.dram_tensor("wkT", [D, PC], F32, kind="ExternalInput")
    wvT_d = nc.dram_tensor("wvT", [D, PC], F32, kind="ExternalInput")
    woT_d = nc.dram_tensor("woT", [D, PC], F32, kind="ExternalInput")
    bq_d = nc.dram_tensor("bq", [PC, 1], F32, kind="ExternalInput")
    bo_d = nc.dram_tensor("bo", [PC, 1], F32, kind="ExternalInput")
    outT_d = nc.dram_tensor("outT", [PC, M], F32, kind="ExternalOutput")
    debug = bool(int(os.environ.get("KERNEL_DEBUG", "0")))
    if debug:
        qT_dbg = nc.dram_tensor("qT_dbg", [128, 2, M], F32, kind="ExternalOutput")
        kT_dbg = nc.dram_tensor("kT_dbg", [128, 2, NSEQ], F32, kind="ExternalOutput")
        v_dbg = nc.dram_tensor(
            "v_dbg", [128, HC, NT, 128], F32, kind="ExternalOutput"
        )
        agin_dbg = nc.dram_tensor("agin_dbg", [PC, M], F32, kind="ExternalOutput")
        agout_dbg = nc.dram_tensor(
            "agout_dbg", [4 * PC, M], F32, kind="ExternalOutput"
        )

    with tile.TileContext(nc) as tc:
        with (
            tc.tile_pool(name="singles", bufs=1) as singles,
            tc.tile_pool(name="dram", bufs=1, space="DRAM") as dram,
        ):
            ident_f = singles.tile([128, 128], F32)
            make_identity(nc, ident_f)
            ident = singles.tile([128, 128], F32R)
            nc.vector.tensor_copy(ident, ident_f)
            bq_sb = singles.tile([128, 2], F32)
            nc.sync.dma_start(
                out=bq_sb, in_=bq_d[:, :].rearrange("(o p) w -> p (o w)", p=128)
            )
            bo_sb = singles.tile([128, 2], F32)
            nc.sync.dma_start(
                out=bo_sb, in_=bo_d[:, :].rearrange("(o p) w -> p (o w)", p=128)
            )

            ag_in = dram.tile([PC, M], F32R)
            ag_out = dram.tile([4 * PC, M], F32R)

            for rep in range(reps):
                _emit_rep(
                    nc, tc, bass, mybir, F32, F32R, AF, rep, single_core,
                    debug and rep == reps - 1,
                    dict(
                        xq_d=xq_d, xk_d=xk_d, xv_d=xv_d, wqT_d=wqT_d,
                        wkT_d=wkT_d, wvT_d=wvT_d, woT_d=woT_d, outT_d=outT_d,
                        ident=ident, bq_sb=bq_sb, bo_sb=bo_sb,
                        ag_in=ag_in, ag_out=ag_out,
                        dbg=dict(
                            qT_dbg=qT_dbg, kT_dbg=kT_dbg, v_dbg=v_dbg,
                            agin_dbg=agin_dbg, agout_dbg=agout_dbg,
                        ) if debug else None,
                    ),
                )
    nc.compile()
    return nc


def _emit_rep(nc, tc, bass, mybir, F32, F32R, AF, rep, single_core, debug, env):
    ident = env["ident"]
    bq_sb, bo_sb = env["bq_sb"], env["bo_sb"]
    ag_in, ag_out = env["ag_in"], env["ag_out"]
    R = f"r{rep}_"

    with tc.tile_pool(name=f"{R}proj_out", bufs=1) as proj_out:
        qT = proj_out.tile([128, 2, M], F32R, name=f"{R}qT")  # [part, pair, m]
        kT = proj_out.tile([128, 2, NSEQ], F32R, name=f"{R}kT")
        # lhsT layout for the AV matmul: col 0 = ones (softmax denom ->
        # PSUM partition 0), cols 1..63 zero, cols 64..127 = V rows
        # (-> PSUM partitions 64..127; DVE 64-partition accesses must
        # start at 0 or 64).
        v_ext = proj_out.tile([128, HC, NT, 128], F32R, name=f"{R}v_ext")
        with tc.tile_pool(name=f"{R}vinit", bufs=1) as vinit:
            ones_sb = vinit.tile([128, HC * NT], F32, name=f"{R}ones")
            nc.vector.memset(ones_sb, 1.0)
            nc.vector.tensor_copy(
                v_ext[:, :, :, 0:1],
                ones_sb[:, :].rearrange("p (h n w) -> p h n w", h=HC, w=1),
            )
            zeros_sb = vinit.tile([128, NT * 63], F32, name=f"{R}zeros")
            nc.vector.memset(zeros_sb, 0.0)
            for h in range(HC):
                nc.vector.tensor_copy(
                    v_ext[:, h, :, 1:64],
                    zeros_sb[:, :].rearrange("p (n w) -> p n w", w=63),
                )

        # ---------------- stage 1: transpose + project ----------------
        with (
            tc.tile_pool(name=f"{R}xT", bufs=1) as xT_pool,
            tc.tile_pool(name=f"{R}nat", bufs=6) as nat_pool,
            tc.tile_pool(name=f"{R}wsb", bufs=2) as w_pool,
            tc.tile_pool(name=f"{R}ps_tr", bufs=4, space="PSUM") as ps_tr,
            tc.tile_pool(name=f"{R}ps_pj", bufs=4, space="PSUM") as ps_pj,
        ):
            for ti, (x_d, w_d) in enumerate(
                [
                    (env["xv_d"], env["wvT_d"]),
                    (env["xk_d"], env["wkT_d"]),
                    (env["xq_d"], env["wqT_d"]),
                ]
            ):
                w_sb = w_pool.tile(
                    [128, CT, PC], F32R, tag="w", name=f"{R}w{ti}"
                )
                nc.gpsimd.dma_start(
                    out=w_sb,
                    in_=w_d[:, :].rearrange("(ct p) c -> p ct c", p=128),
                )
                xT = xT_pool.tile([128, CT, M], F32R, tag="xT", name=f"{R}xT{ti}")
                # transpose x into xT
                for rtg in range(4):
                    nats = []
                    for i in range(4):
                        nt_t = nat_pool.tile(
                            [128, D], F32R, tag="nat", name=f"{R}nat{ti}_{rtg}_{i}"
                        )
                        r0 = (rtg * 4 + i) * 128
                        nc.gpsimd.dma_start(out=nt_t, in_=x_d[r0 : r0 + 128, :])
                        nats.append(nt_t)
                    for ct in range(CT):
                        pst = ps_tr.tile(
                            [128, 512], F32R, tag="pst", name=f"{R}pst{ti}_{rtg}_{ct}"
                        )
                        for i in range(4):
                            nc.tensor.transpose(
                                pst[:, i * 128 : (i + 1) * 128],
                                nats[i][:, ct * 128 : (ct + 1) * 128],
                                ident,
                            )
                        dst = xT[:, ct, rtg * 512 : (rtg + 1) * 512]
                        nc.vector.tensor_copy(dst, pst)
                # project
                if ti > 0:  # K^T / Q^T: [oc on partitions, m free]
                    dst_T = kT if ti == 1 else qT
                    for ot in range(2):
                        for mh in range(2):
                            pss = [
                                ps_pj.tile(
                                    [128, 512], F32, tag="pspj",
                                    name=f"{R}pspj{ti}_{ot}_{mh}_{k}",
                                )
                                for k in range(2)
                            ]
                            for ct in range(CT):
                                for mi in range(2):
                                    m = mh * 2 + mi
                                    nc.tensor.matmul(
                                        pss[mi],
                                        w_sb[:, ct, ot * 128 : (ot + 1) * 128],
                                        xT[:, ct, m * 512 : (m + 1) * 512],
                                        start=(ct == 0),
                                        stop=(ct == CT - 1),
                                    )
                            for mi in range(2):
                                m = mh * 2 + mi
                                dst = dst_T[:, ot, m * 512 : (m + 1) * 512]
                                if ti == 2:
                                    nc.vector.tensor_scalar_add(
                                        dst, pss[mi], bq_sb[:, ot : ot + 1]
                                    )
                                else:
                                    # bk dropped: row-constant logit shift
                                    # cancels in softmax
                                    nc.vector.tensor_copy(dst, pss[mi])
                elif True:  # V natural: [n on partitions, head chans free]
                    for nt in range(NT):
                        psv = ps_pj.tile(
                            [128, 512], F32, tag="pspj", name=f"{R}psv{nt}"
                        )
                        for ct in range(CT):
                            nc.tensor.matmul(
                                psv[:, 0:PC],
                                xT[:, ct, nt * 128 : (nt + 1) * 128],
                                w_sb[:, ct, :],
                                start=(ct == 0),
                                stop=(ct == CT - 1),
                            )
                        nc.vector.tensor_copy(
                            v_ext[:, :, nt, 64 : 64 + DH],
                            psv[:, 0:PC].rearrange("p (h d) -> p h d", h=HC),
                        )

        # ---------------- stage 2: attention ----------------
        with (
            tc.tile_pool(name=f"{R}ps_qk", bufs=2, space="PSUM") as ps_qk,
            tc.tile_pool(name=f"{R}ps_av", bufs=4, space="PSUM") as ps_av,
            tc.tile_pool(name=f"{R}at", bufs=3) as at_pool,
            tc.tile_pool(name=f"{R}eps", bufs=3) as eps_pool,
        ):
            # head pair p = heads (2p, 2p+1): head 2p lives on partitions
            # 0..63, head 2p+1 on 64..127 of the pair's qT/kT tile, so the
            # two K=64 S^T matmuls occupy disjoint PE row groups and run
            # concurrently (row tiling).
            for p in range(2):
                for m in range(MT):
                    ps_o = [
                        ps_av.tile(
                            [128, 512], F32, tag="pso", name=f"{R}pso{p}_{m}_{j}"
                        )
                        for j in range(2)
                    ]
                    for nt in range(NT):
                        ps_s = ps_qk.tile(
                            [128, 1024], F32, tag="pss", name=f"{R}pss{p}_{m}_{nt}"
                        )
                        for j in range(2):
                            base = j * 64
                            nc.tensor.matmul(
                                ps_s[:, j * 512 : (j + 1) * 512],
                                kT[base : base + 64, p, nt * 128 : (nt + 1) * 128],
                                qT[base : base + 64, p, m * 512 : (m + 1) * 512],
                                start=True,
                                stop=True,
                            )
                        at = at_pool.tile(
                            [128, 1024], F32R, tag="at", name=f"{R}at{p}_{m}_{nt}"
                        )
                        nc.scalar.activation(at, ps_s, AF.Exp)
                        for j in range(2):
                            nc.tensor.matmul(
                                ps_o[j],
                                v_ext[:, 2 * p + j, nt, :],
                                at[:, j * 512 : (j + 1) * 512],
                                start=(nt == 0),
                                stop=(nt == NT - 1),
                            )
                    for j in range(2):
                        h = 2 * p + j
                        rec = eps_pool.tile(
                            [1, 512], F32, tag="rec", name=f"{R}rec{p}_{m}_{j}"
                        )
                        nc.vector.reciprocal(rec, ps_o[j][0:1, :])
                        rbc = eps_pool.tile(
                            [128, 512], F32, tag="rbc", name=f"{R}rbc{p}_{m}_{j}"
                        )
                        nc.gpsimd.partition_broadcast(rbc, rec[0:1, :])
                        osc = eps_pool.tile(
                            [128, 512], F32R, tag="osc", name=f"{R}osc{p}_{m}_{j}"
                        )
                        nc.vector.tensor_mul(
                            osc[64:128, :], ps_o[j][64:128, :], rbc[64:128, :]
                        )
                        nc.sync.dma_start(
                            out=ag_in[
                                h * DH : (h + 1) * DH, m * 512 : (m + 1) * 512
                            ],
                            in_=osc[64:128, :],
                        )

        # ---------------- stage 3: allgather + output projection ----
        if single_core:
            # stand-in for the AllGather so TimelineSim (single-core,
            # no collectives) can model the rest of the pipeline
            for rr in range(4):
                nc.sync.dma_start(
                    out=ag_out[rr * PC : (rr + 1) * PC, :], in_=ag_in[:, :]
                )
        else:
            nc.gpsimd.collective_compute(
                "AllGather",
                bass.mybir.AluOpType.bypass,
                replica_groups=[[0, 1, 2, 3], [4, 5, 6, 7]],
                ins=[ag_in[:, :].opt()],
                outs=[ag_out[:, :].opt()],
            )

        if debug:
            d = env["dbg"]
            nc.sync.dma_start(out=d["qT_dbg"][:, :, :], in_=qT[:, :, :].bitcast(F32))
            nc.sync.dma_start(out=d["kT_dbg"][:, :, :], in_=kT[:, :, :].bitcast(F32))
            nc.sync.dma_start(
                out=d["v_dbg"][:, :, :, :], in_=v_ext[:, :, :, :].bitcast(F32)
            )
            nc.gpsimd.dma_start(
                out=d["agin_dbg"][:, :], in_=ag_in[:, :].bitcast(F32)
            )
            nc.gpsimd.dma_start(
                out=d["agout_dbg"][:, :], in_=ag_out[:, :].bitcast(F32)
            )

        with (
            tc.tile_pool(name=f"{R}og", bufs=1) as og_pool,
            tc.tile_pool(name=f"{R}wo", bufs=1) as wo_pool,
            tc.tile_pool(name=f"{R}osb", bufs=4) as osb_pool,
            tc.tile_pool(name=f"{R}ps_o2", bufs=3, space="PSUM") as ps_o2,
        ):
            wo_sb = wo_pool.tile([128, CT, PC], F32R, name=f"{R}wo_sb")
            nc.gpsimd.dma_start(
                out=wo_sb,
                in_=env["woT_d"][:, :].rearrange("(ct p) c -> p ct c", p=128),
            )
            og = og_pool.tile([128, CT, M], F32R, name=f"{R}og")
            for ct in range(CT):
                nc.sync.dma_start(
                    out=og[:, ct, :], in_=ag_out[ct * 128 : (ct + 1) * 128, :]
                )
            for ot in range(2):
                for m in range(MT):
                    po = ps_o2.tile(
                        [128, 512], F32, tag="po", name=f"{R}po{ot}_{m}"
                    )
                    for ct in range(CT):
                        nc.tensor.matmul(
                            po,
                            wo_sb[:, ct, ot * 128 : (ot + 1) * 128],
                            og[:, ct, m * 512 : (m + 1) * 512],
                            start=(ct == 0),
                            stop=(ct == CT - 1),
                        )
                    osb = osb_pool.tile(
                        [128, 512], F32, tag="osb", name=f"{R}osb{ot}_{m}"
                    )
                    nc.scalar.activation(
                        osb, po, AF.Identity, bias=bo_sb[:, ot : ot + 1]
                    )
                    nc.sync.dma_start(
                        out=env["outT_d"][
                            ot * 128 : (ot + 1) * 128, m * 512 : (m + 1) * 512
                        ],
                        in_=osb,
                    )


def _make_in_maps(queries, keys, values, Wq, bq, Wk, bk, Wv, bv, Wo, bo):
    # bv folds through attention (softmax weights sum to 1) and the output
    # projection into an effective output bias; bk shifts every logit in a
    # row equally so softmax cancels it.
    bo_eff = bo + Wo @ bv
    c = np.ascontiguousarray
    in_maps = []
    for core in range(NCORES):
        b, r = core // 4, core % 4
        sl = slice(r * PC, (r + 1) * PC)
        in_maps.append(
            {
                "xq": c(queries[b]),
                "xk": c(keys[b]),
                "xv": c(values[b]),
                "wqT": c(Wq[sl, :].T),
                "wkT": c(Wk[sl, :].T),
                "wvT": c(Wv[sl, :].T),
                "woT": c(Wo.T[:, sl]),
                "bq": c(bq[sl].reshape(PC, 1)),
                "bo": c(bo_eff[sl].reshape(PC, 1)),
            }
        )
    return in_maps


def kernel(queries, keys, values, Wq, bq, Wk, bk, Wv, bv, Wo, bo, _trace=False):
    import concourse.bass_utils as bass_utils

    args = [queries, keys, values, Wq, bq, Wk, bk, Wv, bv, Wo, bo]
    args = [np.asarray(a, dtype=np.float32) for a in args]

    if "nc" not in _CACHE:
        _CACHE["nc"] = _build()
    nc = _CACHE["nc"]

    in_maps = _make_in_maps(*args)
    res = bass_utils.run_bass_kernel_spmd(
        nc, in_maps, core_ids=list(range(NCORES)), trace=_trace
    )
    _CACHE["last_result"] = res

    out = np.empty((B, M, D), dtype=np.float32)
    for core in range(NCORES):
        b, r = core // 4, core % 4
        out[b, :, r * PC : (r + 1) * PC] = res.results[core]["outT"].T
    return out


# revision 16
# speedup vs baseline: 1.0184x; 1.0184x over previous
"""Multi-head attention (B=2, M=N=2048, D=1024, H=16, DH=64) on 8 TRN2 cores.

Sharding: data-parallel over batch (cores 0-3 = batch 0, 4-7 = batch 1),
tensor-parallel over heads within each batch group (4 heads/core).
Per core:
  - PE-transpose queries/keys/values (chan on partitions)
  - f32r projections: Q^T, K^T (64-chan head rows on partitions, pair-packed
    into two 128-partition tiles), V natural (seq on partitions) in an
    extended lhsT layout [ones | zeros | V] (softmax row-sum trick)
  - attention per (head, m-chunk): S^T = K_h Q_h^T per 128-row n-tile ->
    exp (ScalarE, f32r out) -> O^T accumulation in PSUM via lhsT=[1|0|V]
    (partition 0 = softmax denominators, partitions 64..127 = O^T rows);
    normalize with reciprocal + gpsimd partition-broadcast
  - AllGather O^T shards (4-core groups) via internal DRAM bounce
  - output projection: each core computes a 256-wide output-channel slice
    (out^T layout) using its host-sliced Wo^T columns; bias folded in.
Host folds bv/bo into one effective bias (bo + Wo @ bv) and drops bk
(row-constant logit shifts cancel in softmax).
"""

import os

import numpy as np

B, M, NSEQ, D = 2, 2048, 2048, 1024
H, DH = 16, 64
HC = 4                # heads per core
PC = HC * DH          # 256 projected channels per core
CT = D // 128         # 8 contraction tiles
NT = NSEQ // 128      # 16 n-tiles
MT = M // 512         # 4 m-chunks
NCORES = 8

_CACHE = {}


def _build(single_core=False, reps=1):
    import concourse.bass as bass
    import concourse.tile as tile
    from concourse import bacc, mybir
    from concourse.masks import make_identity

    F32 = mybir.dt.float32
    F32R = mybir.dt.float32r
    AF = mybir.ActivationFunctionType

    nc = bacc.Bacc(
        "TRN2",
        target_bir_lowering=False,
        debug=False,
        num_devices=1 if single_core else 8,
    )

    xq_d = nc.dram_tensor("xq", [M, D], F32, kind="ExternalInput")
    xk_d = nc.dram_tensor("xk", [NSEQ, D], F32, kind="ExternalInput")
    xv_d = nc.dram_tensor("xv", [NSEQ, D], F32, kind="ExternalInput")
    wqT_d = nc.dram_tensor("wqT", [D, PC], F32, kind="ExternalInput")
    wkT_d = nc.dram_tensor("wkT", [D, PC], F32, kind="ExternalInput")
    wvT_d = nc.dram_tensor("wvT", [D, PC], F32, kind="ExternalInput")
    woT_d = nc.dram_tensor("woT", [D, PC], F32, kind="ExternalInput")
    bq_d = nc.dram_tensor("bq", [PC, 1], F32, kind="ExternalInput")
    bo_d = nc.dram_tensor("bo", [PC, 1], F32, kind="ExternalInput")
    outT_d = nc.dram_tensor("outT", [PC, M], F32, kind="ExternalOutput")
    debug = bool(int(os.environ.get("KERNEL_DEBUG", "0")))
    if debug:
        qT_dbg = nc.dram_tensor("qT_dbg", [128, 2, M], F32, kind="ExternalOutput")
        kT_dbg = nc.dram_tensor("kT_dbg", [128, 2, NSEQ], F32, kind="ExternalOutput")
        v_dbg = nc.dram_tensor(
            "v_dbg", [128, HC, NT, 128], F32, kind="ExternalOutput"
        )
        agin_dbg = nc.dram_tensor("agin_dbg", [PC, M], F32, kind="ExternalOutput")
        agout_dbg = nc.dram_tensor(
            "agout_dbg", [4 * PC, M], F32, kind="ExternalOutput"
        )

    with tile.TileContext(nc) as tc:
        with (
            tc.tile_pool(name="singles", bufs=1) as singles,
            tc.tile_pool(name="dram", bufs=1, space="DRAM") as dram,
        ):
            ident = singles.tile([128, 128], F32)
            make_identity(nc, ident)
            bq_sb = singles.tile([128, 2], F32)
            nc.sync.dma_start(
                out=bq_sb, in_=bq_d[:, :].rearrange("(o p) w -> p (o w)", p=128)
            )
            bo_sb = singles.tile([128, 2], F32)
            nc.sync.dma_start(
                out=bo_sb, in_=bo_d[:, :].rearrange("(o p) w -> p (o w)", p=128)
            )

            ag_in = dram.tile([PC, M], F32R)
            ag_out = dram.tile([4 * PC, M], F32R)

            for rep in range(reps):
                _emit_rep(
                    nc, tc, bass, mybir, F32, F32R, AF, rep, single_core,
                    debug and rep == reps - 1,
                    dict(
                        xq_d=xq_d, xk_d=xk_d, xv_d=xv_d, wqT_d=wqT_d,
                        wkT_d=wkT_d, wvT_d=wvT_d, woT_d=woT_d, outT_d=outT_d,
                        ident=ident, bq_sb=bq_sb, bo_sb=bo_sb,
                        ag_in=ag_in, ag_out=ag_out,
                        dbg=dict(
                            qT_dbg=qT_dbg, kT_dbg=kT_dbg, v_dbg=v_dbg,
                            agin_dbg=agin_dbg, agout_dbg=agout_dbg,
                        ) if debug else None,
                    ),
                )
    nc.compile()
    return nc


def _emit_rep(nc, tc, bass, mybir, F32, F32R, AF, rep, single_core, debug, env):
    ident = env["ident"]
    bq_sb, bo_sb = env["bq_sb"], env["bo_sb"]
    ag_in, ag_out = env["ag_in"], env["ag_out"]
    R = f"r{rep}_"

    with tc.tile_pool(name=f"{R}proj_out", bufs=1) as proj_out:
        qT = proj_out.tile([128, 2, M], F32R, name=f"{R}qT")  # [part, pair, m]
        kT = proj_out.tile([128, 2, NSEQ], F32R, name=f"{R}kT")
        # lhsT layout for the AV matmul: col 0 = ones (softmax denom ->
        # PSUM partition 0), cols 1..63 zero, cols 64..127 = V rows
        # (-> PSUM partitions 64..127; DVE 64-partition accesses must
        # start at 0 or 64).
        v_ext = proj_out.tile([128, HC, NT, 128], F32R, name=f"{R}v_ext")
        with tc.tile_pool(name=f"{R}vinit", bufs=1) as vinit:
            ones_sb = vinit.tile([128, HC * NT], F32, name=f"{R}ones")
            nc.vector.memset(ones_sb, 1.0)
            nc.vector.tensor_copy(
                v_ext[:, :, :, 0:1],
                ones_sb[:, :].rearrange("p (h n w) -> p h n w", h=HC, w=1),
            )
            zeros_sb = vinit.tile([128, NT * 63], F32, name=f"{R}zeros")
            nc.vector.memset(zeros_sb, 0.0)
            for h in range(HC):
                nc.vector.tensor_copy(
                    v_ext[:, h, :, 1:64],
                    zeros_sb[:, :].rearrange("p (n w) -> p n w", w=63),
                )

        # ---------------- stage 1: transpose + project ----------------
        with (
            tc.tile_pool(name=f"{R}xT", bufs=1) as xT_pool,
            tc.tile_pool(name=f"{R}nat", bufs=6) as nat_pool,
            tc.tile_pool(name=f"{R}wsb", bufs=2) as w_pool,
            tc.tile_pool(name=f"{R}ps_tr", bufs=4, space="PSUM") as ps_tr,
            tc.tile_pool(name=f"{R}ps_pj", bufs=4, space="PSUM") as ps_pj,
        ):
            for ti, (x_d, w_d) in enumerate(
                [
                    (env["xv_d"], env["wvT_d"]),
                    (env["xk_d"], env["wkT_d"]),
                    (env["xq_d"], env["wqT_d"]),
                ]
            ):
                w_sb = w_pool.tile(
                    [128, CT, PC], F32R, tag="w", name=f"{R}w{ti}"
                )
                nc.gpsimd.dma_start(
                    out=w_sb,
                    in_=w_d[:, :].rearrange("(ct p) c -> p ct c", p=128),
                )
                xT = xT_pool.tile([128, CT, M], F32R, tag="xT", name=f"{R}xT{ti}")
                # transpose x into xT
                for rtg in range(4):
                    nats = []
                    for i in range(4):
                        nt_t = nat_pool.tile(
                            [128, D], F32, tag="nat", name=f"{R}nat{ti}_{rtg}_{i}"
                        )
                        r0 = (rtg * 4 + i) * 128
                        nc.sync.dma_start(out=nt_t, in_=x_d[r0 : r0 + 128, :])
                        nats.append(nt_t)
                    for ct in range(CT):
                        pst = ps_tr.tile(
                            [128, 512], F32, tag="pst", name=f"{R}pst{ti}_{rtg}_{ct}"
                        )
                        for i in range(4):
                            nc.tensor.transpose(
                                pst[:, i * 128 : (i + 1) * 128],
                                nats[i][:, ct * 128 : (ct + 1) * 128],
                                ident,
                            )
                        dst = xT[:, ct, rtg * 512 : (rtg + 1) * 512]
                        nc.vector.tensor_copy(dst, pst)
                # project
                if ti > 0:  # K^T / Q^T: [oc on partitions, m free]
                    dst_T = kT if ti == 1 else qT
                    for ot in range(2):
                        for mh in range(2):
                            pss = [
                                ps_pj.tile(
                                    [128, 512], F32, tag="pspj",
                                    name=f"{R}pspj{ti}_{ot}_{mh}_{k}",
                                )
                                for k in range(2)
                            ]
                            for ct in range(CT):
                                for mi in range(2):
                                    m = mh * 2 + mi
                                    nc.tensor.matmul(
                                        pss[mi],
                                        w_sb[:, ct, ot * 128 : (ot + 1) * 128],
                                        xT[:, ct, m * 512 : (m + 1) * 512],
                                        start=(ct == 0),
                                        stop=(ct == CT - 1),
                                    )
                            for mi in range(2):
                                m = mh * 2 + mi
                                dst = dst_T[:, ot, m * 512 : (m + 1) * 512]
                                if ti == 2:
                                    nc.vector.tensor_scalar_add(
                                        dst, pss[mi], bq_sb[:, ot : ot + 1]
                                    )
                                else:
                                    # bk dropped: row-constant logit shift
                                    # cancels in softmax
                                    nc.vector.tensor_copy(dst, pss[mi])
                elif True:  # V natural: [n on partitions, head chans free]
                    for nt in range(NT):
                        psv = ps_pj.tile(
                            [128, 512], F32, tag="pspj", name=f"{R}psv{nt}"
                        )
                        for ct in range(CT):
                            nc.tensor.matmul(
                                psv[:, 0:PC],
                                xT[:, ct, nt * 128 : (nt + 1) * 128],
                                w_sb[:, ct, :],
                                start=(ct == 0),
                                stop=(ct == CT - 1),
                            )
                        nc.vector.tensor_copy(
                            v_ext[:, :, nt, 64 : 64 + DH],
                            psv[:, 0:PC].rearrange("p (h d) -> p h d", h=HC),
                        )

        # ---------------- stage 2: attention ----------------
        with (
            tc.tile_pool(name=f"{R}ps_qk", bufs=2, space="PSUM") as ps_qk,
            tc.tile_pool(name=f"{R}ps_av", bufs=4, space="PSUM") as ps_av,
            tc.tile_pool(name=f"{R}at", bufs=3) as at_pool,
            tc.tile_pool(name=f"{R}eps", bufs=3) as eps_pool,
        ):
            # head pair p = heads (2p, 2p+1): head 2p lives on partitions
            # 0..63, head 2p+1 on 64..127 of the pair's qT/kT tile, so the
            # two K=64 S^T matmuls occupy disjoint PE row groups and run
            # concurrently (row tiling).
            for p in range(2):
                for m in range(MT):
                    ps_o = [
                        ps_av.tile(
                            [128, 512], F32, tag="pso", name=f"{R}pso{p}_{m}_{j}"
                        )
                        for j in range(2)
                    ]
                    for nt in range(NT):
                        ps_s = ps_qk.tile(
                            [128, 1024], F32, tag="pss", name=f"{R}pss{p}_{m}_{nt}"
                        )
                        for j in range(2):
                            base = j * 64
                            nc.tensor.matmul(
                                ps_s[:, j * 512 : (j + 1) * 512],
                                kT[base : base + 64, p, nt * 128 : (nt + 1) * 128],
                                qT[base : base + 64, p, m * 512 : (m + 1) * 512],
                                start=True,
                                stop=True,
                            )
                        at = at_pool.tile(
                            [128, 1024], F32R, tag="at", name=f"{R}at{p}_{m}_{nt}"
                        )
                        nc.scalar.activation(at, ps_s, AF.Exp)
                        for j in range(2):
                            nc.tensor.matmul(
                                ps_o[j],
                                v_ext[:, 2 * p + j, nt, :],
                                at[:, j * 512 : (j + 1) * 512],
                                start=(nt == 0),
                                stop=(nt == NT - 1),
                            )
                    for j in range(2):
                        h = 2 * p + j
                        rec = eps_pool.tile(
                            [1, 512], F32, tag="rec", name=f"{R}rec{p}_{m}_{j}"
                        )
                        nc.vector.reciprocal(rec, ps_o[j][0:1, :])
                        rbc = eps_pool.tile(
                            [128, 512], F32, tag="rbc", name=f"{R}rbc{p}_{m}_{j}"
                        )
                        nc.gpsimd.partition_broadcast(rbc, rec[0:1, :])
                        osc = eps_pool.tile(
                            [128, 512], F32R, tag="osc", name=f"{R}osc{p}_{m}_{j}"
                        )
                        nc.vector.tensor_mul(
                            osc[64:128, :], ps_o[j][64:128, :], rbc[64:128, :]
                        )
                        nc.sync.dma_start(
                            out=ag_in[
                                h * DH : (h + 1) * DH, m * 512 : (m + 1) * 512
                            ],
                            in_=osc[64:128, :],
                        )

        # ---------------- stage 3: allgather + output projection ----
        if single_core:
            # stand-in for the AllGather so TimelineSim (single-core,
            # no collectives) can model the rest of the pipeline
            for rr in range(4):
                nc.sync.dma_start(
                    out=ag_out[rr * PC : (rr + 1) * PC, :], in_=ag_in[:, :]
                )
        else:
            nc.gpsimd.collective_compute(
                "AllGather",
                bass.mybir.AluOpType.bypass,
                replica_groups=[[0, 1, 2, 3], [4, 5, 6, 7]],
                ins=[ag_in[:, :].opt()],
                outs=[ag_out[:, :].opt()],
            )

        if debug:
            d = env["dbg"]
            nc.sync.dma_start(out=d["qT_dbg"][:, :, :], in_=qT[:, :, :].bitcast(F32))
            nc.sync.dma_start(out=d["kT_dbg"][:, :, :], in_=kT[:, :, :].bitcast(F32))
            nc.sync.dma_start(
                out=d["v_dbg"][:, :, :, :], in_=v_ext[:, :, :, :].bitcast(F32)
            )
            nc.gpsimd.dma_start(
                out=d["agin_dbg"][:, :], in_=ag_in[:, :].bitcast(F32)
            )
            nc.gpsimd.dma_start(
                out=d["agout_dbg"][:, :], in_=ag_out[:, :].bitcast(F32)
            )

        with (
            tc.tile_pool(name=f"{R}og", bufs=1) as og_pool,
            tc.tile_pool(name=f"{R}wo", bufs=1) as wo_pool,
            tc.tile_pool(name=f"{R}osb", bufs=4) as osb_pool,
            tc.tile_pool(name=f"{R}ps_o2", bufs=3, space="PSUM") as ps_o2,
        ):
            wo_sb = wo_pool.tile([128, CT, PC], F32R, name=f"{R}wo_sb")
            nc.gpsimd.dma_start(
                out=wo_sb,
                in_=env["woT_d"][:, :].rearrange("(ct p) c -> p ct c", p=128),
            )
            og = og_pool.tile([128, CT, M], F32R, name=f"{R}og")
            for ct in range(CT):
                nc.sync.dma_start(
                    out=og[:, ct, :], in_=ag_out[ct * 128 : (ct + 1) * 128, :]
                )
            for ot in range(2):
                for m in range(MT):
                    po = ps_o2.tile(
                        [128, 512], F32, tag="po", name=f"{R}po{ot}_{m}"
                    )
                    for ct in range(CT):
                        nc.tensor.matmul(
                            po,
                            wo_sb[:, ct, ot * 128 : (ot + 1) * 128],
                            og[:, ct, m * 512 : (m + 1) * 512],
                            start=(ct == 0),
                            stop=(ct == CT - 1),
                        )
                    osb = osb_pool.tile(
                        [128, 512], F32, tag="osb", name=f"{R}osb{ot}_{m}"
                    )
                    nc.scalar.activation(
                        osb, po, AF.Identity, bias=bo_sb[:, ot : ot + 1]
                    )
                    nc.sync.dma_start(
                        out=env["outT_d"][
                            ot * 128 : (ot + 1) * 128, m * 512 : (m + 1) * 512
                        ],
                        in_=osb,
                    )


def _make_in_maps(queries, keys, values, Wq, bq, Wk, bk, Wv, bv, Wo, bo):
    # bv folds through attention (softmax weights sum to 1) and the output
    # projection into an effective output bias; bk shifts every logit in a
    # row equally so softmax cancels it.
    bo_eff = bo + Wo @ bv
    c = np.ascontiguousarray
    in_maps = []
    for core in range(NCORES):
        b, r = core // 4, core % 4
        sl = slice(r * PC, (r + 1) * PC)
        in_maps.append(
            {
                "xq": c(queries[b]),
                "xk": c(keys[b]),
                "xv": c(values[b]),
                "wqT": c(Wq[sl, :].T),
                "wkT": c(Wk[sl, :].T),
                "wvT": c(Wv[sl, :].T),
                "woT": c(Wo.T[:, sl]),
                "bq": c(bq[sl].reshape(PC, 1)),
                "bo": c(bo_eff[sl].reshape(PC, 1)),
            }
        )
    return in_maps


def kernel(queries, keys, values, Wq, bq, Wk, bk, Wv, bv, Wo, bo, _trace=False):
    import concourse.bass_utils as bass_utils

    args = [queries, keys, values, Wq, bq, Wk, bk, Wv, bv, Wo, bo]
    args = [np.asarray(a, dtype=np.float32) for a in args]

    if "nc" not in _CACHE:
        _CACHE["nc"] = _build()
    nc = _CACHE["nc"]

    in_maps = _make_in_maps(*args)
    res = bass_utils.run_bass_kernel_spmd(
        nc, in_maps, core_ids=list(range(NCORES)), trace=_trace
    )
    _CACHE["last_result"] = res

    out = np.empty((B, M, D), dtype=np.float32)
    for core in range(NCORES):
        b, r = core // 4, core % 4
        out[b, :, r * PC : (r + 1) * PC] = res.results[core]["outT"].T
    return out


# revision 19
# speedup vs baseline: 1.2591x; 1.2364x over previous
"""Multi-head attention (B=2, M=N=2048, D=1024, H=16, DH=64) on 8 TRN2 cores.

Sharding: data-parallel over batch (cores 0-3 = batch 0, 4-7 = batch 1),
tensor-parallel over heads within each batch group (4 heads/core).
Per core:
  - PE-transpose queries/keys/values (chan on partitions)
  - f32r projections: Q^T, K^T (64-chan head rows on partitions, pair-packed
    into two 128-partition tiles), V natural (seq on partitions) in an
    extended lhsT layout [ones | zeros | V] (softmax row-sum trick)
  - attention per (head, m-chunk): S^T = K_h Q_h^T per 128-row n-tile ->
    exp (ScalarE, f32r out) -> O^T accumulation in PSUM via lhsT=[1|0|V]
    (partition 0 = softmax denominators, partitions 64..127 = O^T rows);
    normalize with reciprocal + gpsimd partition-broadcast
  - AllGather O^T shards (4-core groups) via internal DRAM bounce
  - output projection: each core computes a 256-wide output-channel slice
    (out^T layout) using its host-sliced Wo^T columns; bias folded in.
Host folds bv/bo into one effective bias (bo + Wo @ bv) and drops bk
(row-constant logit shifts cancel in softmax).
"""

import os

import numpy as np

B, M, NSEQ, D = 2, 2048, 2048, 1024
H, DH = 16, 64
HC = 4                # heads per core
PC = HC * DH          # 256 projected channels per core
CT = D // 128         # 8 contraction tiles
NT = NSEQ // 128      # 16 n-tiles
MT = M // 512         # 4 m-chunks
NCORES = 8

_CACHE = {}


def _build(single_core=False, reps=1):
    import concourse.bass as bass
    import concourse.tile as tile
    from concourse import bacc, mybir
    from concourse.masks import make_identity

    F32 = mybir.dt.float32
    F32R = mybir.dt.float32r
    AF = mybir.ActivationFunctionType

    nc = bacc.Bacc(
        "TRN2",
        target_bir_lowering=False,
        debug=False,
        num_devices=1 if single_core else 8,
    )

    xq_d = nc.dram_tensor("xq", [M, D], F32, kind="ExternalInput")
    xk_d = nc.dram_tensor("xk", [NSEQ, D], F32, kind="ExternalInput")
    xv_d = nc.dram_tensor("xv", [NSEQ, D], F32, kind="ExternalInput")
    wqT_d = nc.dram_tensor("wqT", [D, PC], F32, kind="ExternalInput")
    wkT_d = nc.dram_tensor("wkT", [D, PC], F32, kind="ExternalInput")
    wvT_d = nc.dram_tensor("wvT", [D, PC], F32, kind="ExternalInput")
    woT_d = nc.dram_tensor("woT", [D, PC], F32, kind="ExternalInput")
    bq_d = nc.dram_tensor("bq", [PC, 1], F32, kind="ExternalInput")
    bo_d = nc.dram_tensor("bo", [PC, 1], F32, kind="ExternalInput")
    outT_d = nc.dram_tensor("outT", [PC, M], F32, kind="ExternalOutput")
    debug = bool(int(os.environ.get("KERNEL_DEBUG", "0")))
    if debug:
        qT_dbg = nc.dram_tensor("qT_dbg", [128, 2, M], F32, kind="ExternalOutput")
        kT_dbg = nc.dram_tensor("kT_dbg", [128, 2, NSEQ], F32, kind="ExternalOutput")
        v_dbg = nc.dram_tensor(
            "v_dbg", [128, HC, NT, 128], F32, kind="ExternalOutput"
        )
        agin_dbg = nc.dram_tensor("agin_dbg", [PC, M], F32, kind="ExternalOutput")
        agout_dbg = nc.dram_tensor(
            "agout_dbg", [4 * PC, M], F32, kind="ExternalOutput"
        )

    with tile.TileContext(nc) as tc:
        with (
            tc.tile_pool(name="singles", bufs=1) as singles,
            tc.tile_pool(name="dram", bufs=1, space="DRAM") as dram,
        ):
            ident = singles.tile([128, 128], F32)
            make_identity(nc, ident)
            bq_sb = singles.tile([128, 2], F32)
            nc.sync.dma_start(
                out=bq_sb, in_=bq_d[:, :].rearrange("(o p) w -> p (o w)", p=128)
            )
            bo_sb = singles.tile([128, 2], F32)
            nc.sync.dma_start(
                out=bo_sb, in_=bo_d[:, :].rearrange("(o p) w -> p (o w)", p=128)
            )

            ag_in = dram.tile([MT, PC, 512], F32R)
            ag_out = dram.tile([MT, 4 * PC, 512], F32R)

            for rep in range(reps):
                _emit_rep(
                    nc, tc, bass, mybir, F32, F32R, AF, rep, single_core,
                    debug and rep == reps - 1,
                    dict(
                        xq_d=xq_d, xk_d=xk_d, xv_d=xv_d, wqT_d=wqT_d,
                        wkT_d=wkT_d, wvT_d=wvT_d, woT_d=woT_d, outT_d=outT_d,
                        ident=ident, bq_sb=bq_sb, bo_sb=bo_sb,
                        ag_in=ag_in, ag_out=ag_out,
                        dbg=dict(
                            qT_dbg=qT_dbg, kT_dbg=kT_dbg, v_dbg=v_dbg,
                            agin_dbg=agin_dbg, agout_dbg=agout_dbg,
                        ) if debug else None,
                    ),
                )
    nc.compile()
    return nc


def _emit_rep(nc, tc, bass, mybir, F32, F32R, AF, rep, single_core, debug, env):
    ident = env["ident"]
    bq_sb, bo_sb = env["bq_sb"], env["bo_sb"]
    ag_in, ag_out = env["ag_in"], env["ag_out"]
    R = f"r{rep}_"

    with tc.tile_pool(name=f"{R}proj_out", bufs=1) as proj_out:
        qT = proj_out.tile([128, 2, M], F32R, name=f"{R}qT")  # [part, pair, m]
        kT = proj_out.tile([128, 2, NSEQ], F32R, name=f"{R}kT")
        # lhsT layout for the AV matmul: col 0 = ones (softmax denom ->
        # PSUM partition 0), cols 1..63 zero, cols 64..127 = V rows
        # (-> PSUM partitions 64..127; DVE 64-partition accesses must
        # start at 0 or 64).
        v_ext = proj_out.tile([128, HC, NT, 128], F32R, name=f"{R}v_ext")
        with tc.tile_pool(name=f"{R}vinit", bufs=1) as vinit:
            ones_sb = vinit.tile([128, HC * NT], F32, name=f"{R}ones")
            nc.vector.memset(ones_sb, 1.0)
            nc.vector.tensor_copy(
                v_ext[:, :, :, 0:1],
                ones_sb[:, :].rearrange("p (h n w) -> p h n w", h=HC, w=1),
            )
            zeros_sb = vinit.tile([128, NT * 63], F32, name=f"{R}zeros")
            nc.vector.memset(zeros_sb, 0.0)
            for h in range(HC):
                nc.vector.tensor_copy(
                    v_ext[:, h, :, 1:64],
                    zeros_sb[:, :].rearrange("p (n w) -> p n w", w=63),
                )

        # S^T/exp pools are opened before the stage-1 pools so the
        # attention exp stream can run concurrently with late stage 1
        # (PSUM: pss 2x2 banks + tr 2 + pj 2 = 8).
        ps_qk = tc.tile_pool(name=f"{R}ps_qk", bufs=2, space="PSUM")
        ps_qk.__enter__()
        at_pool = tc.tile_pool(name=f"{R}at", bufs=6)
        at_pool.__enter__()

        # ---------------- stage 1: transpose + project ----------------
        with (
            tc.tile_pool(name=f"{R}xT", bufs=1) as xT_pool,
            tc.tile_pool(name=f"{R}nat", bufs=6) as nat_pool,
            tc.tile_pool(name=f"{R}wsb", bufs=2) as w_pool,
            tc.tile_pool(name=f"{R}ps_tr", bufs=2, space="PSUM") as ps_tr,
            tc.tile_pool(name=f"{R}ps_pj", bufs=2, space="PSUM") as ps_pj,
        ):
            for ti, (x_d, w_d) in enumerate(
                [
                    (env["xk_d"], env["wkT_d"]),
                    (env["xq_d"], env["wqT_d"]),
                    (env["xv_d"], env["wvT_d"]),
                ]
            ):
                w_sb = w_pool.tile(
                    [128, CT, PC], F32R, tag="w", name=f"{R}w{ti}"
                )
                nc.gpsimd.dma_start(
                    out=w_sb,
                    in_=w_d[:, :].rearrange("(ct p) c -> p ct c", p=128),
                )
                xT = xT_pool.tile([128, CT, M], F32R, tag="xT", name=f"{R}xT{ti}")
                # transpose x into xT
                for rtg in range(4):
                    nats = []
                    for i in range(4):
                        nt_t = nat_pool.tile(
                            [128, D], F32, tag="nat", name=f"{R}nat{ti}_{rtg}_{i}"
                        )
                        r0 = (rtg * 4 + i) * 128
                        nc.sync.dma_start(out=nt_t, in_=x_d[r0 : r0 + 128, :])
                        nats.append(nt_t)
                    for ct in range(CT):
                        pst = ps_tr.tile(
                            [128, 512], F32, tag="pst", name=f"{R}pst{ti}_{rtg}_{ct}"
                        )
                        for i in range(4):
                            nc.tensor.transpose(
                                pst[:, i * 128 : (i + 1) * 128],
                                nats[i][:, ct * 128 : (ct + 1) * 128],
                                ident,
                            )
                        dst = xT[:, ct, rtg * 512 : (rtg + 1) * 512]
                        if ct % 2 == 0:
                            nc.vector.tensor_copy(dst, pst)
                        else:
                            nc.scalar.activation(dst, pst, AF.Copy)
                # project
                if ti < 2:  # K^T / Q^T: [oc on partitions, m free]
                    dst_T = kT if ti == 0 else qT
                    for m in range(MT):
                        for ot in range(2):
                            pj = ps_pj.tile(
                                [128, 512], F32, tag="pspj",
                                name=f"{R}pspj{ti}_{m}_{ot}",
                            )
                            for ct in range(CT):
                                nc.tensor.matmul(
                                    pj,
                                    w_sb[:, ct, ot * 128 : (ot + 1) * 128],
                                    xT[:, ct, m * 512 : (m + 1) * 512],
                                    start=(ct == 0),
                                    stop=(ct == CT - 1),
                                )
                            dst = dst_T[:, ot, m * 512 : (m + 1) * 512]
                            if ti == 1:
                                if m % 2 == 0:
                                    nc.vector.tensor_scalar_add(
                                        dst, pj, bq_sb[:, ot : ot + 1]
                                    )
                                else:
                                    nc.scalar.activation(
                                        dst, pj, AF.Identity,
                                        bias=bq_sb[:, ot : ot + 1],
                                    )
                            else:
                                # bk dropped: row-constant logit shift
                                # cancels in softmax
                                if m % 2 == 0:
                                    nc.vector.tensor_copy(dst, pj)
                                else:
                                    nc.scalar.activation(dst, pj, AF.Copy)
                else:  # V natural: [n on partitions, head chans free]
                    for nt in range(NT):
                        psv = ps_pj.tile(
                            [128, 512], F32, tag="pspj", name=f"{R}psv{nt}"
                        )
                        for ct in range(CT):
                            nc.tensor.matmul(
                                psv[:, 0:PC],
                                xT[:, ct, nt * 128 : (nt + 1) * 128],
                                w_sb[:, ct, :],
                                start=(ct == 0),
                                stop=(ct == CT - 1),
                            )
                        nc.vector.tensor_copy(
                            v_ext[:, :, nt, 64 : 64 + DH],
                            psv[:, 0:PC].rearrange("p (h d) -> p h d", h=HC),
                        )

        # ------- stage 2+3: attention with per-m AllGather + out-proj -------
        with (
            tc.tile_pool(name=f"{R}ps_av", bufs=3, space="PSUM") as ps_av,
            tc.tile_pool(name=f"{R}ps_o2", bufs=1, space="PSUM") as ps_o2,
            tc.tile_pool(name=f"{R}eps", bufs=3) as eps_pool,
            tc.tile_pool(name=f"{R}og", bufs=2) as og_pool,
            tc.tile_pool(name=f"{R}wo", bufs=1) as wo_pool,
            tc.tile_pool(name=f"{R}osb", bufs=4) as osb_pool,
        ):
            wo_sb = wo_pool.tile([128, CT, PC], F32R, name=f"{R}wo_sb")
            nc.gpsimd.dma_start(
                out=wo_sb,
                in_=env["woT_d"][:, :].rearrange("(ct p) c -> p ct c", p=128),
            )
            # head pair p = heads (2p, 2p+1): head 2p lives on partitions
            # 0..63, head 2p+1 on 64..127 of the pair's qT/kT tile, so the
            # two K=64 S^T matmuls occupy disjoint PE row groups and run
            # concurrently (row tiling).
            for m in range(MT):
                for p in range(2):
                    ps_o = [
                        ps_av.tile(
                            [128, 512], F32, tag="pso", name=f"{R}pso{m}_{p}_{j}"
                        )
                        for j in range(2)
                    ]
                    for nt in range(NT):
                        ps_s = ps_qk.tile(
                            [128, 1024], F32, tag="pss", name=f"{R}pss{m}_{p}_{nt}"
                        )
                        for j in range(2):
                            base = j * 64
                            nc.tensor.matmul(
                                ps_s[:, j * 512 : (j + 1) * 512],
                                kT[base : base + 64, p, nt * 128 : (nt + 1) * 128],
                                qT[base : base + 64, p, m * 512 : (m + 1) * 512],
                                start=True,
                                stop=True,
                            )
                        at = at_pool.tile(
                            [128, 1024], F32R, tag="at", name=f"{R}at{m}_{p}_{nt}"
                        )
                        nc.scalar.activation(at, ps_s, AF.Exp)
                        for j in range(2):
                            nc.tensor.matmul(
                                ps_o[j],
                                v_ext[:, 2 * p + j, nt, :],
                                at[:, j * 512 : (j + 1) * 512],
                                start=(nt == 0),
                                stop=(nt == NT - 1),
                            )
                    for j in range(2):
                        h = 2 * p + j
                        rec = eps_pool.tile(
                            [1, 512], F32, tag="rec", name=f"{R}rec{m}_{p}_{j}"
                        )
                        nc.vector.reciprocal(rec, ps_o[j][0:1, :])
                        rbc = eps_pool.tile(
                            [128, 512], F32, tag="rbc", name=f"{R}rbc{m}_{p}_{j}"
                        )
                        nc.gpsimd.partition_broadcast(rbc, rec[0:1, :])
                        osc = eps_pool.tile(
                            [128, 512], F32R, tag="osc", name=f"{R}osc{m}_{p}_{j}"
                        )
                        nc.vector.tensor_mul(
                            osc[64:128, :], ps_o[j][64:128, :], rbc[64:128, :]
                        )
                        nc.sync.dma_start(
                            out=ag_in[m, h * DH : (h + 1) * DH, :],
                            in_=osc[64:128, :],
                        )

                # per-m AllGather of the (PC, 512) O^T slice, then this
                # m-chunk's output projection — pipelines under the
                # attention for later m-chunks.
                if single_core:
                    for rr in range(4):
                        nc.sync.dma_start(
                            out=ag_out[m, rr * PC : (rr + 1) * PC, :],
                            in_=ag_in[m, :, :],
                        )
                else:
                    nc.gpsimd.collective_compute(
                        "AllGather",
                        bass.mybir.AluOpType.bypass,
                        replica_groups=[[0, 1, 2, 3], [4, 5, 6, 7]],
                        ins=[ag_in[m, :, :].opt()],
                        outs=[ag_out[m, :, :].opt()],
                    )

                og = og_pool.tile([128, CT, 512], F32R, tag="og", name=f"{R}og{m}")
                for ct in range(CT):
                    nc.sync.dma_start(
                        out=og[:, ct, :],
                        in_=ag_out[m, ct * 128 : (ct + 1) * 128, :],
                    )
                for ot in range(2):
                    po = ps_o2.tile(
                        [128, 512], F32, tag="po", name=f"{R}po{m}_{ot}"
                    )
                    for ct in range(CT):
                        nc.tensor.matmul(
                            po,
                            wo_sb[:, ct, ot * 128 : (ot + 1) * 128],
                            og[:, ct, :],
                            start=(ct == 0),
                            stop=(ct == CT - 1),
                        )
                    osb = osb_pool.tile(
                        [128, 512], F32, tag="osb", name=f"{R}osb{m}_{ot}"
                    )
                    nc.scalar.activation(
                        osb, po, AF.Identity, bias=bo_sb[:, ot : ot + 1]
                    )
                    nc.sync.dma_start(
                        out=env["outT_d"][
                            ot * 128 : (ot + 1) * 128, m * 512 : (m + 1) * 512
                        ],
                        in_=osb,
                    )

        at_pool.__exit__(None, None, None)
        ps_qk.__exit__(None, None, None)

        if debug:
            d = env["dbg"]
            nc.sync.dma_start(out=d["qT_dbg"][:, :, :], in_=qT[:, :, :].bitcast(F32))
            nc.sync.dma_start(out=d["kT_dbg"][:, :, :], in_=kT[:, :, :].bitcast(F32))
            nc.sync.dma_start(
                out=d["v_dbg"][:, :, :, :], in_=v_ext[:, :, :, :].bitcast(F32)
            )
            nc.gpsimd.dma_start(
                out=d["agin_dbg"][:, :], in_=ag_in[:, :, :].bitcast(F32)
            )
            nc.gpsimd.dma_start(
                out=d["agout_dbg"][:, :], in_=ag_out[:, :, :].bitcast(F32)
            )


def _make_in_maps(queries, keys, values, Wq, bq, Wk, bk, Wv, bv, Wo, bo):
    # bv folds through attention (softmax weights sum to 1) and the output
    # projection into an effective output bias; bk shifts every logit in a
    # row equally so softmax cancels it.
    bo_eff = bo + Wo @ bv
    c = np.ascontiguousarray
    in_maps = []
    for core in range(NCORES):
        b, r = core // 4, core % 4
        sl = slice(r * PC, (r + 1) * PC)
        in_maps.append(
            {
                "xq": c(queries[b]),
                "xk": c(keys[b]),
                "xv": c(values[b]),
                "wqT": c(Wq[sl, :].T),
                "wkT": c(Wk[sl, :].T),
                "wvT": c(Wv[sl, :].T),
                "woT": c(Wo.T[:, sl]),
                "bq": c(bq[sl].reshape(PC, 1)),
                "bo": c(bo_eff[sl].reshape(PC, 1)),
            }
        )
    return in_maps


def kernel(queries, keys, values, Wq, bq, Wk, bk, Wv, bv, Wo, bo, _trace=False):
    import concourse.bass_utils as bass_utils

    args = [queries, keys, values, Wq, bq, Wk, bk, Wv, bv, Wo, bo]
    args = [np.asarray(a, dtype=np.float32) for a in args]

    if "nc" not in _CACHE:
        _CACHE["nc"] = _build()
    nc = _CACHE["nc"]

    in_maps = _make_in_maps(*args)
    res = bass_utils.run_bass_kernel_spmd(
        nc, in_maps, core_ids=list(range(NCORES)), trace=_trace
    )
    _CACHE["last_result"] = res

    out = np.empty((B, M, D), dtype=np.float32)
    for core in range(NCORES):
        b, r = core // 4, core % 4
        out[b, :, r * PC : (r + 1) * PC] = res.results[core]["outT"].T
    return out


# revision 23
# speedup vs baseline: 1.6567x; 1.3158x over previous
"""Multi-head attention (B=2, M=N=2048, D=1024, H=16, DH=64) on 8 TRN2 cores.

Sharding: data-parallel over batch (cores 0-3 = batch 0, 4-7 = batch 1),
tensor-parallel over heads within each batch group (4 heads/core). All
matmuls run in f32r (tf32-class PE fast path, 1 cycle/row at free>=256).

Per core:
  - stage 1 (K, then Q, then V): PE-transpose x into chan-on-partition
    layout (128x128 identity transposes, 4 per PSUM bank, copies
    alternating DVE/ScalarE), then project: Q^T/K^T with the 4 heads'
    64-row blocks pair-packed into two 128-partition tiles (head 2p on
    partitions 0..63, head 2p+1 on 64..127 of pair tile p); V in natural
    seq-on-partition layout embedded in an extended AV-lhsT
    [ones | zeros | V] (col 0 ones => softmax denominators land on PSUM
    partition 0; V cols 64..127 => O^T rows on partitions 64..127, the
    only other legal 64-partition DVE base).
  - stage 2, per m-chunk of 512 query rows: for each head pair, 16
    n-tiles: two K=64 S^T = K_h Q_h^T matmuls on disjoint PE row groups
    (concurrent via row tiling) into one 2-bank PSUM tile -> one
    ScalarE Exp (f32r out) -> two AV matmuls accumulating O^T (+denoms)
    in PSUM; normalize via DVE reciprocal + gpsimd partition-broadcast +
    DVE multiply. Each m-chunk's (256, 512) O^T shard is AllGathered
    across the 4-core batch group immediately (4 small collectives
    pipeline under later m-chunks' compute).
  - stage 3 (emitted last => lowest PE priority, fills PE gaps): each
    core computes a 256-wide output-channel slice out^T = Wo_slice^T.T @
    O^T_full per m-chunk from the gathered shards; bias on the
    PSUM->SBUF copy.
Host-side prep: weights are pre-transposed/sliced per core (that is how
rank-dependence enters an otherwise rank-oblivious SPMD program); bv is
folded through attention+projection into bo_eff = bo + Wo @ bv; bk is
dropped (row-constant logit shifts cancel in softmax). Output assembly
is a pure concat/transpose of per-core (256, 2048) out^T slices.
"""

import os

import numpy as np

B, M, NSEQ, D = 2, 2048, 2048, 1024
H, DH = 16, 64
HC = 4                # heads per core
PC = HC * DH          # 256 projected channels per core
CT = D // 128         # 8 contraction tiles
NT = NSEQ // 128      # 16 n-tiles
MT = M // 512         # 4 m-chunks
NCORES = 8

_CACHE = {}


def _build(single_core=False, reps=1):
    import concourse.bass as bass
    import concourse.tile as tile
    from concourse import bacc, mybir
    from concourse.masks import make_identity

    F32 = mybir.dt.float32
    F32R = mybir.dt.float32r
    AF = mybir.ActivationFunctionType

    nc = bacc.Bacc(
        "TRN2",
        target_bir_lowering=False,
        debug=False,
        num_devices=1 if single_core else 8,
    )

    xq_d = nc.dram_tensor("xq", [M, D], F32, kind="ExternalInput")
    xk_d = nc.dram_tensor("xk", [NSEQ, D], F32, kind="ExternalInput")
    xv_d = nc.dram_tensor("xv", [NSEQ, D], F32, kind="ExternalInput")
    wqT_d = nc.dram_tensor("wqT", [D, PC], F32, kind="ExternalInput")
    wkT_d = nc.dram_tensor("wkT", [D, PC], F32, kind="ExternalInput")
    wvT_d = nc.dram_tensor("wvT", [D, PC], F32, kind="ExternalInput")
    woT_d = nc.dram_tensor("woT", [D, PC], F32, kind="ExternalInput")
    bq_d = nc.dram_tensor("bq", [PC, 1], F32, kind="ExternalInput")
    bo_d = nc.dram_tensor("bo", [PC, 1], F32, kind="ExternalInput")
    outT_d = nc.dram_tensor("outT", [PC, M], F32, kind="ExternalOutput")
    debug = bool(int(os.environ.get("KERNEL_DEBUG", "0")))
    if debug:
        qT_dbg = nc.dram_tensor("qT_dbg", [128, 2, M], F32, kind="ExternalOutput")
        kT_dbg = nc.dram_tensor("kT_dbg", [128, 2, NSEQ], F32, kind="ExternalOutput")
        v_dbg = nc.dram_tensor(
            "v_dbg", [128, HC, NT, 128], F32, kind="ExternalOutput"
        )
        agin_dbg = nc.dram_tensor("agin_dbg", [PC, M], F32, kind="ExternalOutput")
        agout_dbg = nc.dram_tensor(
            "agout_dbg", [4 * PC, M], F32, kind="ExternalOutput"
        )

    with tile.TileContext(nc) as tc:
        with (
            tc.tile_pool(name="singles", bufs=1) as singles,
            tc.tile_pool(name="dram", bufs=1, space="DRAM") as dram,
        ):
            ident = singles.tile([128, 128], F32)
            make_identity(nc, ident)
            bq_sb = singles.tile([128, 2], F32)
            nc.sync.dma_start(
                out=bq_sb, in_=bq_d[:, :].rearrange("(o p) w -> p (o w)", p=128)
            )
            bo_sb = singles.tile([128, 2], F32)
            nc.sync.dma_start(
                out=bo_sb, in_=bo_d[:, :].rearrange("(o p) w -> p (o w)", p=128)
            )

            ag_in = dram.tile([MT, PC, 512], F32R)
            ag_out = dram.tile([MT, 4 * PC, 512], F32R)

            for rep in range(reps):
                _emit_rep(
                    nc, tc, bass, mybir, F32, F32R, AF, rep, single_core,
                    debug and rep == reps - 1,
                    dict(
                        xq_d=xq_d, xk_d=xk_d, xv_d=xv_d, wqT_d=wqT_d,
                        wkT_d=wkT_d, wvT_d=wvT_d, woT_d=woT_d, outT_d=outT_d,
                        ident=ident, bq_sb=bq_sb, bo_sb=bo_sb,
                        ag_in=ag_in, ag_out=ag_out,
                        dbg=dict(
                            qT_dbg=qT_dbg, kT_dbg=kT_dbg, v_dbg=v_dbg,
                            agin_dbg=agin_dbg, agout_dbg=agout_dbg,
                        ) if debug else None,
                    ),
                )
    nc.compile()
    return nc


def _emit_rep(nc, tc, bass, mybir, F32, F32R, AF, rep, single_core, debug, env):
    ident = env["ident"]
    bq_sb, bo_sb = env["bq_sb"], env["bo_sb"]
    ag_in, ag_out = env["ag_in"], env["ag_out"]
    R = f"r{rep}_"

    with tc.tile_pool(name=f"{R}proj_out", bufs=1) as proj_out:
        qT = proj_out.tile([128, 2, M], F32R, name=f"{R}qT")  # [part, pair, m]
        kT = proj_out.tile([128, 2, NSEQ], F32R, name=f"{R}kT")
        # lhsT layout for the AV matmul: col 0 = ones (softmax denom ->
        # PSUM partition 0), cols 1..63 zero, cols 64..127 = V rows
        # (-> PSUM partitions 64..127; DVE 64-partition accesses must
        # start at 0 or 64).
        v_ext = proj_out.tile([128, HC, NT, 128], F32R, name=f"{R}v_ext")
        with tc.tile_pool(name=f"{R}vinit", bufs=1) as vinit:
            ones_sb = vinit.tile([128, HC * NT], F32, name=f"{R}ones")
            nc.vector.memset(ones_sb, 1.0)
            nc.vector.tensor_copy(
                v_ext[:, :, :, 0:1],
                ones_sb[:, :].rearrange("p (h n w) -> p h n w", h=HC, w=1),
            )
            zeros_sb = vinit.tile([128, NT * 63], F32, name=f"{R}zeros")
            nc.vector.memset(zeros_sb, 0.0)
            for h in range(HC):
                nc.vector.tensor_copy(
                    v_ext[:, h, :, 1:64],
                    zeros_sb[:, :].rearrange("p (n w) -> p n w", w=63),
                )

        # ---------------- stage 1: transpose + project ----------------
        with (
            tc.tile_pool(name=f"{R}xT", bufs=1) as xT_pool,
            tc.tile_pool(name=f"{R}nat", bufs=6) as nat_pool,
            tc.tile_pool(name=f"{R}wsb", bufs=2) as w_pool,
            tc.tile_pool(name=f"{R}ps_tr", bufs=4, space="PSUM") as ps_tr,
            tc.tile_pool(name=f"{R}ps_pj", bufs=3, space="PSUM") as ps_pj,
        ):
            for ti, (x_d, w_d) in enumerate(
                [
                    (env["xk_d"], env["wkT_d"]),
                    (env["xq_d"], env["wqT_d"]),
                    (env["xv_d"], env["wvT_d"]),
                ]
            ):
                w_sb = w_pool.tile(
                    [128, CT, PC], F32R, tag="w", name=f"{R}w{ti}"
                )
                nc.gpsimd.dma_start(
                    out=w_sb,
                    in_=w_d[:, :].rearrange("(ct p) c -> p ct c", p=128),
                )
                xT = xT_pool.tile([128, CT, M], F32R, tag="xT", name=f"{R}xT{ti}")
                # transpose x into xT
                for rtg in range(4):
                    nats = []
                    for i in range(4):
                        nt_t = nat_pool.tile(
                            [128, D], F32, tag="nat", name=f"{R}nat{ti}_{rtg}_{i}"
                        )
                        r0 = (rtg * 4 + i) * 128
                        nc.sync.dma_start(out=nt_t, in_=x_d[r0 : r0 + 128, :])
                        nats.append(nt_t)
                    for ct in range(CT):
                        pst = ps_tr.tile(
                            [128, 512], F32, tag="pst", name=f"{R}pst{ti}_{rtg}_{ct}"
                        )
                        for i in range(4):
                            nc.tensor.transpose(
                                pst[:, i * 128 : (i + 1) * 128],
                                nats[i][:, ct * 128 : (ct + 1) * 128],
                                ident,
                            )
                        dst = xT[:, ct, rtg * 512 : (rtg + 1) * 512]
                        if ct % 2 == 0:
                            nc.vector.tensor_copy(dst, pst)
                        else:
                            nc.scalar.activation(dst, pst, AF.Copy)
                # project
                if ti < 2:  # K^T / Q^T: [oc on partitions, m free]
                    dst_T = kT if ti == 0 else qT
                    for m in range(MT):
                        for ot in range(2):
                            pj = ps_pj.tile(
                                [128, 512], F32, tag="pspj",
                                name=f"{R}pspj{ti}_{m}_{ot}",
                            )
                            for ct in range(CT):
                                nc.tensor.matmul(
                                    pj,
                                    w_sb[:, ct, ot * 128 : (ot + 1) * 128],
                                    xT[:, ct, m * 512 : (m + 1) * 512],
                                    start=(ct == 0),
                                    stop=(ct == CT - 1),
                                )
                            dst = dst_T[:, ot, m * 512 : (m + 1) * 512]
                            if ti == 1:
                                if m % 2 == 0:
                                    nc.vector.tensor_scalar_add(
                                        dst, pj, bq_sb[:, ot : ot + 1]
                                    )
                                else:
                                    nc.scalar.activation(
                                        dst, pj, AF.Identity,
                                        bias=bq_sb[:, ot : ot + 1],
                                    )
                            else:
                                # bk dropped: row-constant logit shift
                                # cancels in softmax
                                if m % 2 == 0:
                                    nc.vector.tensor_copy(dst, pj)
                                else:
                                    nc.scalar.activation(dst, pj, AF.Copy)
                else:  # V natural: [n on partitions, head chans free]
                    for nt in range(NT):
                        psv = ps_pj.tile(
                            [128, 512], F32, tag="pspj", name=f"{R}psv{nt}"
                        )
                        for ct in range(CT):
                            nc.tensor.matmul(
                                psv[:, 0:PC],
                                xT[:, ct, nt * 128 : (nt + 1) * 128],
                                w_sb[:, ct, :],
                                start=(ct == 0),
                                stop=(ct == CT - 1),
                            )
                        nc.vector.tensor_copy(
                            v_ext[:, :, nt, 64 : 64 + DH],
                            psv[:, 0:PC].rearrange("p (h d) -> p h d", h=HC),
                        )

        # ------- stage 2+3: attention with per-m AllGather + out-proj -------
        with (
            tc.tile_pool(name=f"{R}ps_qk", bufs=2, space="PSUM") as ps_qk,
            tc.tile_pool(name=f"{R}ps_av", bufs=4, space="PSUM") as ps_av,
            tc.tile_pool(name=f"{R}at", bufs=4) as at_pool,
            tc.tile_pool(name=f"{R}eps", bufs=3) as eps_pool,
            tc.tile_pool(name=f"{R}og", bufs=2) as og_pool,
            tc.tile_pool(name=f"{R}wo", bufs=1) as wo_pool,
            tc.tile_pool(name=f"{R}osb", bufs=4) as osb_pool,
        ):
            wo_sb = wo_pool.tile([128, CT, PC], F32R, name=f"{R}wo_sb")
            nc.gpsimd.dma_start(
                out=wo_sb,
                in_=env["woT_d"][:, :].rearrange("(ct p) c -> p ct c", p=128),
            )
            # head pair p = heads (2p, 2p+1): head 2p lives on partitions
            # 0..63, head 2p+1 on 64..127 of the pair's qT/kT tile, so the
            # two K=64 S^T matmuls occupy disjoint PE row groups and run
            # concurrently (row tiling).
            for m in range(MT):
                for p in range(2):
                    ps_o = [
                        ps_av.tile(
                            [128, 512], F32, tag="pso", name=f"{R}pso{m}_{p}_{j}"
                        )
                        for j in range(2)
                    ]
                    for nt in range(NT):
                        ps_s = ps_qk.tile(
                            [128, 1024], F32, tag="pss", name=f"{R}pss{m}_{p}_{nt}"
                        )
                        for j in range(2):
                            base = j * 64
                            nc.tensor.matmul(
                                ps_s[:, j * 512 : (j + 1) * 512],
                                kT[base : base + 64, p, nt * 128 : (nt + 1) * 128],
                                qT[base : base + 64, p, m * 512 : (m + 1) * 512],
                                start=True,
                                stop=True,
                            )
                        at = at_pool.tile(
                            [128, 1024], F32R, tag="at", name=f"{R}at{m}_{p}_{nt}"
                        )
                        nc.scalar.activation(at, ps_s, AF.Exp)
                        for j in range(2):
                            nc.tensor.matmul(
                                ps_o[j],
                                v_ext[:, 2 * p + j, nt, :],
                                at[:, j * 512 : (j + 1) * 512],
                                start=(nt == 0),
                                stop=(nt == NT - 1),
                            )
                    for j in range(2):
                        h = 2 * p + j
                        rec = eps_pool.tile(
                            [1, 512], F32, tag="rec", name=f"{R}rec{m}_{p}_{j}"
                        )
                        nc.vector.reciprocal(rec, ps_o[j][0:1, :])
                        rbc = eps_pool.tile(
                            [128, 512], F32, tag="rbc", name=f"{R}rbc{m}_{p}_{j}"
                        )
                        nc.gpsimd.partition_broadcast(rbc, rec[0:1, :])
                        osc = eps_pool.tile(
                            [128, 512], F32R, tag="osc", name=f"{R}osc{m}_{p}_{j}"
                        )
                        nc.vector.tensor_mul(
                            osc[64:128, :], ps_o[j][64:128, :], rbc[64:128, :]
                        )
                        nc.sync.dma_start(
                            out=ag_in[m, h * DH : (h + 1) * DH, :],
                            in_=osc[64:128, :],
                        )

                # per-m AllGather of the (PC, 512) O^T slice, then this
                # m-chunk's output projection — pipelines under the
                # attention for later m-chunks.
                if single_core:
                    for rr in range(4):
                        nc.sync.dma_start(
                            out=ag_out[m, rr * PC : (rr + 1) * PC, :],
                            in_=ag_in[m, :, :],
                        )
                else:
                    nc.gpsimd.collective_compute(
                        "AllGather",
                        bass.mybir.AluOpType.bypass,
                        replica_groups=[[0, 1, 2, 3], [4, 5, 6, 7]],
                        ins=[ag_in[m, :, :].opt()],
                        outs=[ag_out[m, :, :].opt()],
                    )


            # output projection last in program order => lowest PE priority:
            # its matmuls fill PE gaps instead of delaying the S^T/exp
            # stream. og loads still chase each per-m AllGather via DMA.
            for m in range(MT):
                og = og_pool.tile([128, CT, 512], F32R, tag="og", name=f"{R}og{m}")
                for ct in range(CT):
                    nc.sync.dma_start(
                        out=og[:, ct, :],
                        in_=ag_out[m, ct * 128 : (ct + 1) * 128, :],
                    )
                for ot in range(2):
                    po = ps_av.tile(
                        [128, 512], F32, tag="pso", name=f"{R}po{m}_{ot}"
                    )
                    for ct in range(CT):
                        nc.tensor.matmul(
                            po,
                            wo_sb[:, ct, ot * 128 : (ot + 1) * 128],
                            og[:, ct, :],
                            start=(ct == 0),
                            stop=(ct == CT - 1),
                        )
                    osb = osb_pool.tile(
                        [128, 512], F32, tag="osb", name=f"{R}osb{m}_{ot}"
                    )
                    nc.scalar.activation(
                        osb, po, AF.Identity, bias=bo_sb[:, ot : ot + 1]
                    )
                    nc.sync.dma_start(
                        out=env["outT_d"][
                            ot * 128 : (ot + 1) * 128, m * 512 : (m + 1) * 512
                        ],
                        in_=osb,
                    )

        if debug:
            d = env["dbg"]
            nc.sync.dma_start(out=d["qT_dbg"][:, :, :], in_=qT[:, :, :].bitcast(F32))
            nc.sync.dma_start(out=d["kT_dbg"][:, :, :], in_=kT[:, :, :].bitcast(F32))
            nc.sync.dma_start(
                out=d["v_dbg"][:, :, :, :], in_=v_ext[:, :, :, :].bitcast(F32)
            )
            nc.gpsimd.dma_start(
                out=d["agin_dbg"][:, :], in_=ag_in[:, :, :].bitcast(F32)
            )
            nc.gpsimd.dma_start(
                out=d["agout_dbg"][:, :], in_=ag_out[:, :, :].bitcast(F32)
            )


def _make_in_maps(queries, keys, values, Wq, bq, Wk, bk, Wv, bv, Wo, bo):
    # bv folds through attention (softmax weights sum to 1) and the output
    # projection into an effective output bias; bk shifts every logit in a
    # row equally so softmax cancels it.
    bo_eff = bo + Wo @ bv
    c = np.ascontiguousarray
    in_maps = []
    for core in range(NCORES):
        b, r = core // 4, core % 4
        sl = slice(r * PC, (r + 1) * PC)
        in_maps.append(
            {
                "xq": c(queries[b]),
                "xk": c(keys[b]),
                "xv": c(values[b]),
                "wqT": c(Wq[sl, :].T),
                "wkT": c(Wk[sl, :].T),
                "wvT": c(Wv[sl, :].T),
                "woT": c(Wo.T[:, sl]),
                "bq": c(bq[sl].reshape(PC, 1)),
                "bo": c(bo_eff[sl].reshape(PC, 1)),
            }
        )
    return in_maps


def kernel(queries, keys, values, Wq, bq, Wk, bk, Wv, bv, Wo, bo, _trace=False):
    import concourse.bass_utils as bass_utils

    args = [queries, keys, values, Wq, bq, Wk, bk, Wv, bv, Wo, bo]
    args = [np.asarray(a, dtype=np.float32) for a in args]

    if "nc" not in _CACHE:
        _CACHE["nc"] = _build()
    nc = _CACHE["nc"]

    in_maps = _make_in_maps(*args)
    res = bass_utils.run_bass_kernel_spmd(
        nc, in_maps, core_ids=list(range(NCORES)), trace=_trace
    )
    _CACHE["last_result"] = res

    out = np.empty((B, M, D), dtype=np.float32)
    for core in range(NCORES):
        b, r = core // 4, core % 4
        out[b, :, r * PC : (r + 1) * PC] = res.results[core]["outT"].T
    return out


# revision 24
# speedup vs baseline: 2.4748x; 1.4938x over previous
"""Multi-head attention (B=2, M=N=2048, D=1024, H=16, DH=64) on 8 TRN2 cores.

Sharding: data-parallel over batch (cores 0-3 = batch 0, 4-7 = batch 1),
tensor-parallel over heads within each batch group (4 heads/core). All
matmuls run in f32r (tf32-class PE fast path, 1 cycle/row at free>=256).

Per core:
  - stage 1 (K, then Q, then V): PE-transpose x into chan-on-partition
    layout (128x128 identity transposes, 4 per PSUM bank, copies
    alternating DVE/ScalarE), then project: Q^T/K^T with the 4 heads'
    64-row blocks pair-packed into two 128-partition tiles (head 2p on
    partitions 0..63, head 2p+1 on 64..127 of pair tile p); V in natural
    seq-on-partition layout embedded in an extended AV-lhsT
    [ones | zeros | V] (col 0 ones => softmax denominators land on PSUM
    partition 0; V cols 64..127 => O^T rows on partitions 64..127, the
    only other legal 64-partition DVE base).
  - stage 2, per m-chunk of 512 query rows: for each head pair, 16
    n-tiles: two K=64 S^T = K_h Q_h^T matmuls on disjoint PE row groups
    (concurrent via row tiling) into one 2-bank PSUM tile -> one
    ScalarE Exp (f32r out) -> two AV matmuls accumulating O^T (+denoms)
    in PSUM; normalize via DVE reciprocal + gpsimd partition-broadcast +
    DVE multiply. Each m-chunk's (256, 512) O^T shard is AllGathered
    across the 4-core batch group immediately (4 small collectives
    pipeline under later m-chunks' compute).
  - stage 3 (emitted last => lowest PE priority, fills PE gaps): each
    core computes a 256-wide output-channel slice out^T = Wo_slice^T.T @
    O^T_full per m-chunk from the gathered shards; bias on the
    PSUM->SBUF copy.
Host-side prep: weights are pre-transposed/sliced per core (that is how
rank-dependence enters an otherwise rank-oblivious SPMD program); bv is
folded through attention+projection into bo_eff = bo + Wo @ bv; bk is
dropped (row-constant logit shifts cancel in softmax). Output assembly
is a pure concat/transpose of per-core (256, 2048) out^T slices.
"""

import os

import numpy as np

B, M, NSEQ, D = 2, 2048, 2048, 1024
H, DH = 16, 64
HC = 4                # heads per core
PC = HC * DH          # 256 projected channels per core
CT = D // 128         # 8 contraction tiles
NT = NSEQ // 128      # 16 n-tiles
MT = M // 512         # 4 m-chunks
NCORES = 8

_CACHE = {}


def _build(single_core=False, reps=1):
    import concourse.bass as bass
    import concourse.tile as tile
    from concourse import bacc, mybir
    from concourse.masks import make_identity

    F32 = mybir.dt.float32
    F32R = mybir.dt.float32r
    AF = mybir.ActivationFunctionType

    nc = bacc.Bacc(
        "TRN2",
        target_bir_lowering=False,
        debug=False,
        num_devices=1 if single_core else 8,
    )

    xq_d = nc.dram_tensor("xq", [M, D], F32, kind="ExternalInput")
    xk_d = nc.dram_tensor("xk", [NSEQ, D], F32, kind="ExternalInput")
    xv_d = nc.dram_tensor("xv", [NSEQ, D], F32, kind="ExternalInput")
    wqT_d = nc.dram_tensor("wqT", [D, PC], F32, kind="ExternalInput")
    wkT_d = nc.dram_tensor("wkT", [D, PC], F32, kind="ExternalInput")
    wvT_d = nc.dram_tensor("wvT", [D, PC], F32, kind="ExternalInput")
    woT_d = nc.dram_tensor("woT", [D, PC], F32, kind="ExternalInput")
    bq_d = nc.dram_tensor("bq", [PC, 1], F32, kind="ExternalInput")
    bo_d = nc.dram_tensor("bo", [PC, 1], F32, kind="ExternalInput")
    outT_d = nc.dram_tensor("outT", [PC, M], F32, kind="ExternalOutput")
    debug = bool(int(os.environ.get("KERNEL_DEBUG", "0")))
    if debug:
        qT_dbg = nc.dram_tensor("qT_dbg", [128, 2, M], F32, kind="ExternalOutput")
        kT_dbg = nc.dram_tensor("kT_dbg", [128, 2, NSEQ], F32, kind="ExternalOutput")
        v_dbg = nc.dram_tensor(
            "v_dbg", [128, HC, NT, 128], F32, kind="ExternalOutput"
        )
        agin_dbg = nc.dram_tensor("agin_dbg", [PC, M], F32, kind="ExternalOutput")
        agout_dbg = nc.dram_tensor(
            "agout_dbg", [4 * PC, M], F32, kind="ExternalOutput"
        )

    with tile.TileContext(nc) as tc:
        with (
            tc.tile_pool(name="singles", bufs=1) as singles,
            tc.tile_pool(name="dram", bufs=1, space="DRAM") as dram,
        ):
            ident = singles.tile([128, 128], F32)
            make_identity(nc, ident)
            bq_sb = singles.tile([128, 2], F32)
            nc.sync.dma_start(
                out=bq_sb, in_=bq_d[:, :].rearrange("(o p) w -> p (o w)", p=128)
            )
            bo_sb = singles.tile([128, 2], F32)
            nc.sync.dma_start(
                out=bo_sb, in_=bo_d[:, :].rearrange("(o p) w -> p (o w)", p=128)
            )

            ag_in = dram.tile([MT, PC, 512], F32R)
            ag_out = dram.tile([MT, 4 * PC, 512], F32R)

            for rep in range(reps):
                _emit_rep(
                    nc, tc, bass, mybir, F32, F32R, AF, rep, single_core,
                    debug and rep == reps - 1,
                    dict(
                        xq_d=xq_d, xk_d=xk_d, xv_d=xv_d, wqT_d=wqT_d,
                        wkT_d=wkT_d, wvT_d=wvT_d, woT_d=woT_d, outT_d=outT_d,
                        ident=ident, bq_sb=bq_sb, bo_sb=bo_sb,
                        ag_in=ag_in, ag_out=ag_out,
                        dbg=dict(
                            qT_dbg=qT_dbg, kT_dbg=kT_dbg, v_dbg=v_dbg,
                            agin_dbg=agin_dbg, agout_dbg=agout_dbg,
                        ) if debug else None,
                    ),
                )
    nc.compile()
    return nc


def _emit_rep(nc, tc, bass, mybir, F32, F32R, AF, rep, single_core, debug, env):
    ident = env["ident"]
    bq_sb, bo_sb = env["bq_sb"], env["bo_sb"]
    ag_in, ag_out = env["ag_in"], env["ag_out"]
    R = f"r{rep}_"

    with tc.tile_pool(name=f"{R}proj_out", bufs=1) as proj_out:
        qT = proj_out.tile([128, 2, M], F32R, name=f"{R}qT")  # [part, pair, m]
        kT = proj_out.tile([128, 2, NSEQ], F32R, name=f"{R}kT")
        # lhsT layout for the AV matmul: col 0 = ones (softmax denom ->
        # PSUM partition 0), cols 1..63 zero, cols 64..127 = V rows
        # (-> PSUM partitions 64..127; DVE 64-partition accesses must
        # start at 0 or 64).
        v_ext = proj_out.tile([128, HC, NT, 128], F32R, name=f"{R}v_ext")
        with tc.tile_pool(name=f"{R}vinit", bufs=1) as vinit:
            ones_sb = vinit.tile([128, HC * NT], F32, name=f"{R}ones")
            nc.vector.memset(ones_sb, 1.0)
            nc.vector.tensor_copy(
                v_ext[:, :, :, 0:1],
                ones_sb[:, :].rearrange("p (h n w) -> p h n w", h=HC, w=1),
            )
            zeros_sb = vinit.tile([128, NT * 63], F32, name=f"{R}zeros")
            nc.vector.memset(zeros_sb, 0.0)
            for h in range(HC):
                nc.vector.tensor_copy(
                    v_ext[:, h, :, 1:64],
                    zeros_sb[:, :].rearrange("p (n w) -> p n w", w=63),
                )

        # ---------------- stage 1: transpose + project ----------------
        with (
            tc.tile_pool(name=f"{R}xT", bufs=1) as xT_pool,
            tc.tile_pool(name=f"{R}nat", bufs=8) as nat_pool,
            tc.tile_pool(name=f"{R}wsb", bufs=2) as w_pool,
            tc.tile_pool(name=f"{R}ps_tr", bufs=4, space="PSUM") as ps_tr,
            tc.tile_pool(name=f"{R}ps_pj", bufs=3, space="PSUM") as ps_pj,
        ):
            for ti, (x_d, w_d) in enumerate(
                [
                    (env["xk_d"], env["wkT_d"]),
                    (env["xq_d"], env["wqT_d"]),
                    (env["xv_d"], env["wvT_d"]),
                ]
            ):
                w_sb = w_pool.tile(
                    [128, CT, PC], F32R, tag="w", name=f"{R}w{ti}"
                )
                nc.gpsimd.dma_start(
                    out=w_sb,
                    in_=w_d[:, :].rearrange("(ct p) c -> p ct c", p=128),
                )
                xT = xT_pool.tile([128, CT, M], F32R, tag="xT", name=f"{R}xT{ti}")
                # transpose x into xT
                for rtg in range(4):
                    nats = []
                    for i in range(4):
                        nt_t = nat_pool.tile(
                            [128, D], F32, tag="nat", name=f"{R}nat{ti}_{rtg}_{i}"
                        )
                        r0 = (rtg * 4 + i) * 128
                        nc.sync.dma_start(out=nt_t, in_=x_d[r0 : r0 + 128, :])
                        nats.append(nt_t)
                    for ct in range(CT):
                        pst = ps_tr.tile(
                            [128, 512], F32, tag="pst", name=f"{R}pst{ti}_{rtg}_{ct}"
                        )
                        for i in range(4):
                            nc.tensor.transpose(
                                pst[:, i * 128 : (i + 1) * 128],
                                nats[i][:, ct * 128 : (ct + 1) * 128],
                                ident,
                            )
                        dst = xT[:, ct, rtg * 512 : (rtg + 1) * 512]
                        if ct % 2 == 0:
                            nc.vector.tensor_copy(dst, pst)
                        else:
                            nc.scalar.activation(dst, pst, AF.Copy)
                # project
                if ti < 2:  # K^T / Q^T: [oc on partitions, m free]
                    dst_T = kT if ti == 0 else qT
                    for m in range(MT):
                        for ot in range(2):
                            pj = ps_pj.tile(
                                [128, 512], F32, tag="pspj",
                                name=f"{R}pspj{ti}_{m}_{ot}",
                            )
                            for ct in range(CT):
                                nc.tensor.matmul(
                                    pj,
                                    w_sb[:, ct, ot * 128 : (ot + 1) * 128],
                                    xT[:, ct, m * 512 : (m + 1) * 512],
                                    start=(ct == 0),
                                    stop=(ct == CT - 1),
                                )
                            dst = dst_T[:, ot, m * 512 : (m + 1) * 512]
                            if ti == 1:
                                if m % 2 == 0:
                                    nc.vector.tensor_scalar_add(
                                        dst, pj, bq_sb[:, ot : ot + 1]
                                    )
                                else:
                                    nc.scalar.activation(
                                        dst, pj, AF.Identity,
                                        bias=bq_sb[:, ot : ot + 1],
                                    )
                            else:
                                # bk dropped: row-constant logit shift
                                # cancels in softmax
                                if m % 2 == 0:
                                    nc.vector.tensor_copy(dst, pj)
                                else:
                                    nc.scalar.activation(dst, pj, AF.Copy)
                else:  # V natural: [n on partitions, head chans free]
                    for nt in range(NT):
                        psv = ps_pj.tile(
                            [128, 512], F32, tag="pspj", name=f"{R}psv{nt}"
                        )
                        for ct in range(CT):
                            nc.tensor.matmul(
                                psv[:, 0:PC],
                                xT[:, ct, nt * 128 : (nt + 1) * 128],
                                w_sb[:, ct, :],
                                start=(ct == 0),
                                stop=(ct == CT - 1),
                            )
                        nc.vector.tensor_copy(
                            v_ext[:, :, nt, 64 : 64 + DH],
                            psv[:, 0:PC].rearrange("p (h d) -> p h d", h=HC),
                        )

        # ------- stage 2+3: attention with per-m AllGather + out-proj -------
        with (
            tc.tile_pool(name=f"{R}ps_qk", bufs=2, space="PSUM") as ps_qk,
            tc.tile_pool(name=f"{R}ps_av", bufs=4, space="PSUM") as ps_av,
            tc.tile_pool(name=f"{R}at", bufs=6) as at_pool,
            tc.tile_pool(name=f"{R}eps", bufs=4) as eps_pool,
            tc.tile_pool(name=f"{R}og", bufs=3) as og_pool,
            tc.tile_pool(name=f"{R}wo", bufs=1) as wo_pool,
            tc.tile_pool(name=f"{R}osb", bufs=4) as osb_pool,
        ):
            wo_sb = wo_pool.tile([128, CT, PC], F32R, name=f"{R}wo_sb")
            nc.gpsimd.dma_start(
                out=wo_sb,
                in_=env["woT_d"][:, :].rearrange("(ct p) c -> p ct c", p=128),
            )
            # head pair p = heads (2p, 2p+1): head 2p lives on partitions
            # 0..63, head 2p+1 on 64..127 of the pair's qT/kT tile, so the
            # two K=64 S^T matmuls occupy disjoint PE row groups and run
            # concurrently (row tiling).
            for m in range(MT):
                for p in range(2):
                    ps_o = [
                        ps_av.tile(
                            [128, 512], F32, tag="pso", name=f"{R}pso{m}_{p}_{j}"
                        )
                        for j in range(2)
                    ]
                    for nt in range(NT):
                        ps_s = ps_qk.tile(
                            [128, 1024], F32, tag="pss", name=f"{R}pss{m}_{p}_{nt}"
                        )
                        for j in range(2):
                            base = j * 64
                            nc.tensor.matmul(
                                ps_s[:, j * 512 : (j + 1) * 512],
                                kT[base : base + 64, p, nt * 128 : (nt + 1) * 128],
                                qT[base : base + 64, p, m * 512 : (m + 1) * 512],
                                start=True,
                                stop=True,
                            )
                        at = at_pool.tile(
                            [128, 1024], F32R, tag="at", name=f"{R}at{m}_{p}_{nt}"
                        )
                        nc.scalar.activation(at, ps_s, AF.Exp)
                        for j in range(2):
                            nc.tensor.matmul(
                                ps_o[j],
                                v_ext[:, 2 * p + j, nt, :],
                                at[:, j * 512 : (j + 1) * 512],
                                start=(nt == 0),
                                stop=(nt == NT - 1),
                            )
                    for j in range(2):
                        h = 2 * p + j
                        rec = eps_pool.tile(
                            [1, 512], F32, tag="rec", name=f"{R}rec{m}_{p}_{j}"
                        )
                        nc.vector.reciprocal(rec, ps_o[j][0:1, :])
                        rbc = eps_pool.tile(
                            [128, 512], F32, tag="rbc", name=f"{R}rbc{m}_{p}_{j}"
                        )
                        nc.gpsimd.partition_broadcast(rbc, rec[0:1, :])
                        osc = eps_pool.tile(
                            [128, 512], F32R, tag="osc", name=f"{R}osc{m}_{p}_{j}"
                        )
                        nc.vector.tensor_mul(
                            osc[64:128, :], ps_o[j][64:128, :], rbc[64:128, :]
                        )
                        nc.sync.dma_start(
                            out=ag_in[m, h * DH : (h + 1) * DH, :],
                            in_=osc[64:128, :],
                        )

                # per-m AllGather of the (PC, 512) O^T slice, then this
                # m-chunk's output projection — pipelines under the
                # attention for later m-chunks.
                if single_core:
                    for rr in range(4):
                        nc.sync.dma_start(
                            out=ag_out[m, rr * PC : (rr + 1) * PC, :],
                            in_=ag_in[m, :, :],
                        )
                else:
                    nc.gpsimd.collective_compute(
                        "AllGather",
                        bass.mybir.AluOpType.bypass,
                        replica_groups=[[0, 1, 2, 3], [4, 5, 6, 7]],
                        ins=[ag_in[m, :, :].opt()],
                        outs=[ag_out[m, :, :].opt()],
                    )


            # output projection last in program order => lowest PE priority:
            # its matmuls fill PE gaps instead of delaying the S^T/exp
            # stream. og loads still chase each per-m AllGather via DMA.
            for m in range(MT):
                og = og_pool.tile([128, CT, 512], F32R, tag="og", name=f"{R}og{m}")
                for ct in range(CT):
                    nc.sync.dma_start(
                        out=og[:, ct, :],
                        in_=ag_out[m, ct * 128 : (ct + 1) * 128, :],
                    )
                for ot in range(2):
                    po = ps_av.tile(
                        [128, 512], F32, tag="pso", name=f"{R}po{m}_{ot}"
                    )
                    for ct in range(CT):
                        nc.tensor.matmul(
                            po,
                            wo_sb[:, ct, ot * 128 : (ot + 1) * 128],
                            og[:, ct, :],
                            start=(ct == 0),
                            stop=(ct == CT - 1),
                        )
                    osb = osb_pool.tile(
                        [128, 512], F32, tag="osb", name=f"{R}osb{m}_{ot}"
                    )
                    nc.scalar.activation(
                        osb, po, AF.Identity, bias=bo_sb[:, ot : ot + 1]
                    )
                    nc.sync.dma_start(
                        out=env["outT_d"][
                            ot * 128 : (ot + 1) * 128, m * 512 : (m + 1) * 512
                        ],
                        in_=osb,
                    )

        if debug:
            d = env["dbg"]
            nc.sync.dma_start(out=d["qT_dbg"][:, :, :], in_=qT[:, :, :].bitcast(F32))
            nc.sync.dma_start(out=d["kT_dbg"][:, :, :], in_=kT[:, :, :].bitcast(F32))
            nc.sync.dma_start(
                out=d["v_dbg"][:, :, :, :], in_=v_ext[:, :, :, :].bitcast(F32)
            )
            nc.gpsimd.dma_start(
                out=d["agin_dbg"][:, :], in_=ag_in[:, :, :].bitcast(F32)
            )
            nc.gpsimd.dma_start(
                out=d["agout_dbg"][:, :], in_=ag_out[:, :, :].bitcast(F32)
            )


def _make_in_maps(queries, keys, values, Wq, bq, Wk, bk, Wv, bv, Wo, bo):
    # bv folds through attention (softmax weights sum to 1) and the output
    # projection into an effective output bias; bk shifts every logit in a
    # row equally so softmax cancels it.
    bo_eff = bo + Wo @ bv
    c = np.ascontiguousarray
    in_maps = []
    for core in range(NCORES):
        b, r = core // 4, core % 4
        sl = slice(r * PC, (r + 1) * PC)
        in_maps.append(
            {
                "xq": c(queries[b]),
                "xk": c(keys[b]),
                "xv": c(values[b]),
                "wqT": c(Wq[sl, :].T),
                "wkT": c(Wk[sl, :].T),
                "wvT": c(Wv[sl, :].T),
                "woT": c(Wo.T[:, sl]),
                "bq": c(bq[sl].reshape(PC, 1)),
                "bo": c(bo_eff[sl].reshape(PC, 1)),
            }
        )
    return in_maps


def kernel(queries, keys, values, Wq, bq, Wk, bk, Wv, bv, Wo, bo, _trace=False):
    import concourse.bass_utils as bass_utils

    args = [queries, keys, values, Wq, bq, Wk, bk, Wv, bv, Wo, bo]
    args = [np.asarray(a, dtype=np.float32) for a in args]

    if "nc" not in _CACHE:
        _CACHE["nc"] = _build()
    nc = _CACHE["nc"]

    in_maps = _make_in_maps(*args)
    res = bass_utils.run_bass_kernel_spmd(
        nc, in_maps, core_ids=list(range(NCORES)), trace=_trace
    )
    _CACHE["last_result"] = res

    out = np.empty((B, M, D), dtype=np.float32)
    for core in range(NCORES):
        b, r = core // 4, core % 4
        out[b, :, r * PC : (r + 1) * PC] = res.results[core]["outT"].T
    return out
